# revision 10
# baseline (speedup 1.0000x reference)
"""Trainium2 Bass kernel for ConsistentSelfAttentionTile.

Reference semantics: T=449 overlapping 64-token tiles; each tile attends to
352 KV tokens = 288 sampled (from a 9x replication of the tile) + the tile
itself; outputs overlap-add, then divide by overlap counts.

Algebraic collapse (verified vs the jax reference):
  * rep[:, idx, :] == tile[:, idx % 64, :], so the sampled KV tokens are tile
    rows with integer multiplicities m_t[w] = 1 + #{s : idx[t,s] % 64 == w}.
  * Per-tile Q/K/V are slices of the full-sequence projections, so all
    per-tile 64x64 score blocks are diagonal blocks of one banded 512x512
    score matrix S (band |i-j| <= 63).
  * S itself collapses: S = x G x^T + ones (x) g with G = Wq^T Wk and
    g = bq Wk, both host-precomputed (weight-only folding). Wk/Wq never ship
    to the device and the K projection disappears. bk only shifts rows
    (softmax-invariant): drop.
  * The device computes S TRANSPOSED: ST = x_band (QG)^T with the g bias
    already folded into QG before its transpose, so exp(ST) = E^T directly
    and no E transpose is ever needed (everything downstream consumes E^T).
  * No rowmax subtraction: |S| <= ~50 so e^S spans ~e^{+-50}, comfortably
    inside fp32/bf16 exponent range; every tensor carrying e^S-scaled values
    (E, W, U) is bf16/fp32. A row-constant shift cancels exactly in E/Z, so
    dropping the max is exact.
  * With E = exp(S), Cm[j,t] = m_t[j-t] (banded):
        Z = Cm^T E^T;  W = maskw * (1/Z);  U = Cm W;  out = (E^T o U)^T V
    maskw bakes in the valid-tile mask and the 1/counts overlap division.
  * bv is folded into the output PSUM as a rank-1 ones (x) bv matmul (the
    attention weights sum to 1 after the counts division, so this is exact).
  * Cm^T is transposed on device (PE transpose) instead of shipped.

Sharding: 8 cores = 2 batches x 4 row-chunks of 128 output rows. Each core
computes its 128 rows end-to-end from a 256-column band of the input (no
cross-core communication).

Schedule: 6 dummy matmuls on a zeroed SBUF tile warm the PE clock gate
while the input DMA lands. Input streams over the two HWDGE queues as 2
pieces each, the second piece chained on the first's completion
(concurrent DMAs on one queue interleave across the shared SDMA-engine
pool, so chaining is the only way to give the score-chain bytes priority).
tile_wait_until hints pin the Tile scheduler to the measured arrival times
so it cannot hoist Wv-gated V matmuls ahead of the score chain. The output
is computed and evacuated in column halves so its DMA starts earlier.

"""

import os
import sys

import numpy as np

try:
    import ml_dtypes
except ImportError:
    ml_dtypes = None

for _p in ("/opt/trn_rl_repo",):
    if _p not in sys.path and os.path.isdir(_p):
        sys.path.insert(0, _p)

B, N, C, W = 2, 512, 512, 64
T = N - W + 1          # 449 tiles
RCH = 128              # output rows per core
NCORES = 8
BAND = 256             # per-core j/t band width (columns [r0-64, r0+192))
KC = C // 128          # 4 contraction chunks
JC = BAND // 128       # 2 band chunks

# blob16 layout (2-byte elements per partition), grouped by DMA piece:
#   A1 (sync):   G01 | cm | mw
#   B1 (scalar): xt | G23 | idf | idb   (lands first: QG starts on k2,k3)
#   A2 (sync):   wv01      (chained on A1)
#   B2 (scalar): wv23      (chained on B1)
OFF_G01 = 0                      # [128, 2, 512] fp16  G chunks 0,1
OFF_CM = OFF_G01 + 2 * C         # [128, 2, 256] bf16 (count ints: exact)
OFF_MW = OFF_CM + JC * BAND      # [128, 2, 128] fp16 mask/counts
END_A1 = OFF_MW + JC * RCH
OFF_XT = END_A1                  # [128, 4, 256] fp16  x^T band chunks
OFF_G23 = OFF_XT + KC * BAND     # [128, 2, 512] fp16  G chunks 2,3
OFF_IDF = OFF_G23 + 2 * C        # [128, 128] fp16 identity
OFF_IDB = OFF_IDF + 128          # [128, 128] bf16 identity (bitcast)
END_B1 = OFF_IDB + 128
OFF_WV01 = END_B1                # [128, 2, 512] fp16  Wv^T chunks 0,1
END_A2 = OFF_WV01 + 2 * C
OFF_WV23 = END_A2                # [128, 2, 512] fp16  Wv^T chunks 2,3
F16 = OFF_WV23 + 2 * C

# misc row blob [1, 1152] fp16: g (bq Wk) | bv | ones
MISC_G = 0
MISC_BV = C
MISC_ONES = 2 * C
F_MISC = 2 * C + 128

N_WARM = 9             # dummy matmuls to ungate the PE clock

_CACHE = {}


def _slim_drain_and_barrier(self, tick_clock, wait_clock):
    """Cheaper TileContext exit: drain only the DMA-queue clocks, one
    sem-only barrier, then reset the Tile semaphores."""
    from concourse.vector_clock import ScopedClock, VectorClock
    from concourse.tile_scheduler import dmasw_start_idx, N_PROCS

    g = tick_clock.global_clock
    dma_clock = VectorClock()
    for idx in range(dmasw_start_idx, N_PROCS):
        t = g.peek_next(idx) - 1
        if t > 0:
            dma_clock.require_at_least(idx, t)
    drain_inst = self.nc.sync.drain()
    wait_clock.add_sem_waits(drain_inst.ins, ScopedClock({None: dma_clock}))
    self.nc.all_engine_barrier(sem_only=True)
    popped = self.nc._tile_sem_poison_stack.pop()
    assert popped is self._sem_poison
    self.nc.clear_and_free_semaphores(list(self.sems.allocated().values()))


def _build_program():
    import concourse.bacc as bacc
    import concourse.mybir as mybir
    import concourse.tile as tile

    fp16 = mybir.dt.float16
    fp32 = mybir.dt.float32
    # Skip Bass's preamble all-engine barrier: all real cross-engine deps
    # here are Tile semaphores, and the one preamble const AP we read (fp32
    # 0.0 for the exp bias) is memset long before the exp fires.
    orig_aeb = bacc.Bacc.all_engine_barrier

    def _noop_aeb(self, *, sem_only=False):
        return None

    bacc.Bacc.all_engine_barrier = _noop_aeb
    try:
        nc = bacc.Bacc("TRN2", target_bir_lowering=False, debug=False)
    finally:
        bacc.Bacc.all_engine_barrier = orig_aeb

    b16_d = nc.declare_dram_parameter("blob16", [128, F16], fp16, isOutput=False)
    misc_d = nc.declare_dram_parameter("misc", [1, F_MISC], fp16, isOutput=False)
    out_d = nc.declare_dram_parameter("out", [RCH, C], fp32, isOutput=True)

    orig_dab = tile.TileContext._drain_and_barrier
    tile.TileContext._drain_and_barrier = _slim_drain_and_barrier
    try:
        _emit_body(nc, tile, mybir, b16_d, misc_d, out_d)
    finally:
        tile.TileContext._drain_and_barrier = orig_dab

    nc.compile()
    return nc


def _emit_body(nc, tile, mybir, b16_d, misc_d, out_d):
    from concourse.tile_rust import add_dep_helper

    fp32 = mybir.dt.float32
    fp16 = mybir.dt.float16
    bf16 = mybir.dt.bfloat16
    COPY = mybir.ActivationFunctionType.Copy

    with tile.TileContext(nc) as tc:

        def at(us):
            """Schedule hint: don't start the instructions in this block
            before `us` microseconds (relative to kernel-body start)."""
            return tc.tile_wait_until(us / 1000.0)

        with (
            tc.tile_pool(name="consts", bufs=1) as consts,
            tc.tile_pool(name="work", bufs=1) as work,
            tc.tile_pool(name="psum", bufs=1, space="PSUM") as psum,
        ):
            b16 = consts.tile([128, F16], fp16)
            misc = consts.tile([1, F_MISC], fp16)
            warm = work.tile([128, 512], bf16)

            # PE warm-up operand: zeros so nothing downstream can see junk.
            nc.gpsimd.memset(warm[:, :], 0)
            nc.gpsimd.dma_start(out=misc[:, :], in_=misc_d[:, :])

            # ---- input DMA: 2 HWDGE queues x 2 pieces, chained in-queue ----
            a1 = nc.sync.dma_start(
                out=b16[:, 0:END_A1], in_=b16_d[:, 0:END_A1])
            b1 = nc.scalar.dma_start(
                out=b16[:, END_A1:END_B1], in_=b16_d[:, END_A1:END_B1])
            a2 = nc.sync.dma_start(
                out=b16[:, END_B1:END_A2], in_=b16_d[:, END_B1:END_A2])
            add_dep_helper(a2.ins, a1.ins, True, "input DMA priority chain")
            b2 = nc.scalar.dma_start(
                out=b16[:, END_A2:F16], in_=b16_d[:, END_A2:F16])
            add_dep_helper(b2.ins, b1.ins, True, "input DMA priority chain")

            # ---- SBUF views ----
            xt_sb = b16[:, OFF_XT:OFF_XT + KC * BAND].rearrange(
                "p (k j) -> p k j", k=KC)
            g01 = b16[:, OFF_G01:OFF_G01 + 2 * C].rearrange(
                "p (k j) -> p k j", k=2)
            g23 = b16[:, OFF_G23:OFF_G23 + 2 * C].rearrange(
                "p (k j) -> p k j", k=2)
            wv01 = b16[:, OFF_WV01:OFF_WV01 + 2 * C].rearrange(
                "p (k j) -> p k j", k=2)
            wv23 = b16[:, OFF_WV23:OFF_WV23 + 2 * C].rearrange(
                "p (k j) -> p k j", k=2)
            g_chunk = [g01[:, 0, :], g01[:, 1, :], g23[:, 0, :], g23[:, 1, :]]
            wv_chunk = [wv01[:, 0, :], wv01[:, 1, :],
                        wv23[:, 0, :], wv23[:, 1, :]]
            idf = b16[:, OFF_IDF:OFF_IDF + 128]
            idb = b16[:, OFF_IDB:OFF_IDB + 128].bitcast(bf16)
            cm_sb = b16[:, OFF_CM:OFF_CM + JC * BAND].bitcast(bf16).rearrange(
                "p (k t) -> p k t", k=JC)
            mw_sb = b16[:, OFF_MW:OFF_MW + JC * RCH].rearrange(
                "p (k r) -> p k r", k=JC)
            g_row = misc[0:1, MISC_G:MISC_G + C]
            bv_row = misc[0:1, MISC_BV:MISC_BV + C]
            ones1 = misc[0:1, MISC_ONES:MISC_ONES + 128]

            # ---- PE clock-gate warm-up (runs while the DMA lands) ----
            ps_aux = psum.tile([128, 512], fp32, tag="ps_aux", bufs=1)
            for _ in range(N_WARM):
                nc.tensor.matmul(
                    ps_aux, lhsT=warm[:, 0:128], rhs=warm[:, :],
                    start=True, stop=True,
                )

            # ---- QG = x G + ones (x) g   [r 128, c2 512] ----
            # accumulation starts on chunks 2,3 (their G rides the
            # earlier-landing B1 piece together with x)
            ps_qg = psum.tile([128, C], fp32, tag="ps_big", bufs=2)
            with at(5.0):
                for i, k in enumerate((2, 3, 0, 1)):
                    nc.tensor.matmul(
                        ps_qg,
                        lhsT=xt_sb[:, k, 64:64 + RCH],
                        rhs=g_chunk[k],
                        start=(i == 0),
                        stop=False,
                    )
                nc.tensor.matmul(
                    ps_qg, lhsT=ones1, rhs=g_row, start=False, stop=True)
            qg_sb = work.tile([128, C], fp16)
            with at(6.3):
                nc.vector.tensor_copy(out=qg_sb, in_=ps_qg)

            # QG^T chunks [c2 128, r 128] via PE transpose (g bias rides
            # along: it was accumulated into QG before the transpose)
            ps_qgt = psum.tile([128, KC, RCH], fp16, tag="ps_t", bufs=1)
            with at(7.0):
                for m in range(KC):
                    nc.tensor.transpose(
                        ps_qgt[:, m, :], qg_sb[:, m * 128:(m + 1) * 128], idf)
            qgt_sb = work.tile([128, KC, RCH], fp16)
            with at(7.7):
                nc.vector.tensor_copy(out=qgt_sb, in_=ps_qgt)

            # ---- transposed scores ST[j 256, r 128] and E^T = exp(ST) ----
            ps_st = psum.tile([128, JC, RCH], fp32, tag="ps_st", bufs=1)
            with at(8.1):
                for jc in range(JC):
                    for k in range(KC):
                        nc.tensor.matmul(
                            ps_st[:, jc, :],
                            lhsT=xt_sb[:, k, jc * 128:(jc + 1) * 128],
                            rhs=qgt_sb[:, k, :],
                            start=(k == 0),
                            stop=(k == KC - 1),
                        )
            et_sb = work.tile([128, JC, RCH], bf16)
            with at(9.3):
                nc.scalar.activation(
                    out=et_sb, in_=ps_st,
                    func=mybir.ActivationFunctionType.Exp,
                    bias=0.0, scale=1.0,
                )

            # ---- V[jc][j 128, c 512] = x Wv (bv folded into out) ----
            v_sb = work.tile([128, JC, C], fp16)
            ps_v0 = psum.tile([128, C], fp32, tag="ps_big", bufs=2)
            with at(9.4):
                for k in range(KC):
                    nc.tensor.matmul(
                        ps_v0, lhsT=xt_sb[:, k, 0:128], rhs=wv_chunk[k],
                        start=(k == 0), stop=(k == KC - 1),
                    )

            # Cm^T on device: 4 PE transposes of the cm blocks, slotted
            # into the Tensor gap while Scalar runs the exp
            ps_cmt = psum.tile([128, JC, BAND], bf16, tag="ps_aux", bufs=1)
            with at(9.5):
                for tch in range(JC):
                    for jc in range(JC):
                        nc.tensor.transpose(
                            ps_cmt[:, tch, jc * 128:(jc + 1) * 128],
                            cm_sb[:, jc, tch * 128:(tch + 1) * 128],
                            idb,
                        )
            cmt_sb = work.tile([128, JC, BAND], bf16)
            with at(10.0):
                nc.scalar.activation(out=cmt_sb, in_=ps_cmt, func=COPY)

            # Z[t 128, r 128] per tch = sum_j Cm[j,t] E^T[j,r]
            ps_z = psum.tile([128, JC, RCH], fp32, tag="ps_z", bufs=1)
            with at(9.9):
                for tch in range(JC):
                    for jc in range(JC):
                        nc.tensor.matmul(
                            ps_z[:, tch, :],
                            lhsT=cm_sb[:, jc, tch * 128:(tch + 1) * 128],
                            rhs=et_sb[:, jc, :],
                            start=(jc == 0),
                            stop=(jc == JC - 1),
                        )
            with at(10.8):
                nc.scalar.activation(out=v_sb[:, 0, :], in_=ps_v0, func=COPY)

            # W[t, r] = maskw / Z  (values span e^{+-50}: keep bf16/fp32)
            rz_sb = work.tile([128, JC, RCH], fp32)
            w_sb = work.tile([128, JC, RCH], bf16)
            with at(10.5):
                nc.vector.reciprocal_approx_fast(out=rz_sb, in_=ps_z)
                nc.vector.tensor_mul(w_sb, rz_sb, mw_sb)

            # V jc=1 fills the Tensor gap while Vector runs the W path
            ps_v1 = psum.tile([128, C], fp32, tag="ps_big", bufs=2)
            with at(10.7):
                for k in range(KC):
                    nc.tensor.matmul(
                        ps_v1, lhsT=xt_sb[:, k, 128:256], rhs=wv_chunk[k],
                        start=(k == 0), stop=(k == KC - 1),
                    )

            # U[j 128, r 128] per jc = sum_t Cm^T[t,j] W[t,r];  A = E^T o U
            ps_u = psum.tile([128, JC, RCH], fp32, tag="ps_u", bufs=1)
            with at(11.5):
                for jc in range(JC):
                    for tch in range(JC):
                        nc.tensor.matmul(
                            ps_u[:, jc, :],
                            lhsT=cmt_sb[:, tch, jc * 128:(jc + 1) * 128],
                            rhs=w_sb[:, tch, :],
                            start=(tch == 0),
                            stop=(tch == JC - 1),
                        )
            with at(12.1):
                nc.scalar.activation(out=v_sb[:, 1, :], in_=ps_v1, func=COPY)
            a_sb = work.tile([128, JC, RCH], fp16)
            with at(12.1):
                nc.vector.tensor_mul(a_sb, ps_u, et_sb)

            # out[r 128, c 512] = sum_j A[j,r] V[j,c] + ones (x) bv,
            # computed and evacuated in column halves so the output DMA
            # starts while the second half is still in the PE
            ps_o = psum.tile([128, C], fp32, tag="ps_big", bufs=2)
            o_sb = work.tile([128, C], fp32)
            with at(12.6):
                for jc in range(JC):
                    nc.tensor.matmul(
                        ps_o,
                        lhsT=a_sb[:, jc, :],
                        rhs=v_sb[:, jc, :],
                        start=(jc == 0),
                        stop=False,
                    )
                nc.tensor.matmul(
                    ps_o, lhsT=ones1, rhs=bv_row, start=False, stop=True)
            with at(13.4):
                nc.vector.tensor_copy(out=o_sb[:, 0:256], in_=ps_o[:, 0:256])
                nc.sync.dma_start(out=out_d[:, 0:256], in_=o_sb[:, 0:256])
            with at(14.0):
                nc.scalar.activation(
                    out=o_sb[:, 256:512], in_=ps_o[:, 256:512], func=COPY)
                nc.scalar.dma_start(
                    out=out_d[:, 256:512], in_=o_sb[:, 256:512])


def _pack128(arr):
    """[n*128, f] row-chunked -> [128, n*f] (chunk-major along free axis)."""
    n = arr.shape[0] // 128
    return np.ascontiguousarray(
        arr.reshape(n, 128, -1).transpose(1, 0, 2).reshape(128, -1)
    )


def _host_prep(image_features, Wq, bq, Wk, bk, Wv, bv, sample_idx):
    """Build the 8 per-core input blobs (pure index/layout work plus
    weight-only constant folding)."""
    x = np.asarray(image_features, np.float32)
    sample_idx = np.asarray(sample_idx)
    Wq = np.asarray(Wq, np.float32)
    Wk = np.asarray(Wk, np.float32)
    Wv = np.asarray(Wv, np.float32)
    bq = np.asarray(bq, np.float32)
    bv = np.asarray(bv, np.float32)

    # score-collapse: S = x G x^T + ones (x) g   (bk drops: row shift)
    G = (Wq.T @ Wk).astype(np.float16)
    g = (bq @ Wk).astype(np.float16)

    # per-tile multiplicities -> banded count matrix Cm[j, t] = m_t[j - t]
    mod = (sample_idx % W).astype(np.int64)                  # [T, S]
    m = np.zeros((T, W), np.float32)
    np.add.at(m, (np.arange(T)[:, None], mod), 1.0)
    m += 1.0
    Cm = np.zeros((N, N), np.float32)
    rows = np.arange(T)
    for w in range(W):
        Cm[rows + w, rows] = m[:, w]

    pos = np.arange(N)
    counts = (np.minimum(pos, N - W) - np.maximum(pos - W + 1, 0) + 1)

    # padded versions for uniform band slicing
    XTp = np.zeros((B, C, N + 2 * 64), np.float16)
    for b in range(B):
        XTp[b, :, 64:64 + N] = x[b].T.astype(np.float16)
    Cmp = np.zeros((N + 2 * 64, N + 2 * 64), np.float32)
    Cmp[64:64 + N, 64:64 + N] = Cm

    g_p = _pack128(G.astype(np.float16))                     # [128, 4*512]
    wvt_p = _pack128(Wv.T.astype(np.float16))                # [128, 4*512]

    misc = np.zeros((1, F_MISC), np.float16)
    misc[0, MISC_G:MISC_G + C] = g
    misc[0, MISC_BV:MISC_BV + C] = bv
    misc[0, MISC_ONES:MISC_ONES + 128] = 1.0

    in_maps = []
    for core in range(NCORES):
        b, rc = divmod(core, NCORES // B)
        r0 = rc * RCH
        xt = XTp[b, :, r0:r0 + BAND]
        cm = np.ascontiguousarray(Cmp[r0:r0 + BAND, r0:r0 + BAND])
        # all-zero columns (padded t) would give Z=0 -> inf*0 = NaN on
        # device; a diagonal 1 keeps Z finite there and is masked out of W
        zero_cols = ~cm.any(axis=0)
        cm[zero_cols, zero_cols] = 1.0
        tl = np.arange(BAND)
        rl = np.arange(RCH)
        tg = r0 - 64 + tl
        rg = r0 + rl
        d = rg[None, :] - tg[:, None]
        valid = (d >= 0) & (d <= W - 1) & (tg[:, None] >= 0) & (tg[:, None] <= T - 1)
        maskw = np.where(
            valid, 1.0 / counts[rg][None, :], 0.0
        ).astype(np.float16)

        b16 = np.zeros((128, F16), np.float16)
        b16[:, OFF_XT:OFF_XT + KC * BAND] = _pack128(xt)
        b16[:, OFF_G01:OFF_G01 + 2 * C] = g_p[:, 0:2 * C]
        b16[:, OFF_G23:OFF_G23 + 2 * C] = g_p[:, 2 * C:4 * C]
        b16[:, OFF_WV01:OFF_WV01 + 2 * C] = wvt_p[:, 0:2 * C]
        b16[:, OFF_WV23:OFF_WV23 + 2 * C] = wvt_p[:, 2 * C:4 * C]
        b16[:, OFF_MW:OFF_MW + JC * RCH] = _pack128(maskw)
        b16[:, OFF_IDF:OFF_IDF + 128] = np.eye(128, dtype=np.float16)
        # bf16-bit segments written through a uint16 view of the fp16 buffer
        b16v = b16.view(np.uint16)
        b16v[:, OFF_CM:OFF_CM + JC * BAND] = _pack128(
            cm.astype(ml_dtypes.bfloat16)).view(np.uint16)
        b16v[:, OFF_IDB:OFF_IDB + 128] = np.eye(
            128, dtype=ml_dtypes.bfloat16).view(np.uint16)

        in_maps.append({"blob16": b16, "misc": misc})
    return in_maps


def run_on_cores(in_maps, trace=False, trace_cores=None):
    from concourse.bass_utils import run_bass_kernel_spmd

    if "nc" not in _CACHE:
        _CACHE["nc"] = _build_program()
    nc = _CACHE["nc"]
    return run_bass_kernel_spmd(
        nc, in_maps, list(range(NCORES)), trace=trace,
        trace_cores=(trace_cores or [0]) if trace else None,
    )


def kernel(image_features, Wq, bq, Wk, bk, Wv, bv, sample_idx):
    in_maps = _host_prep(image_features, Wq, bq, Wk, bk, Wv, bv, sample_idx)
    res = run_on_cores(in_maps, trace=False)
    out = np.empty((B, N, C), np.float32)
    for core in range(NCORES):
        b, rc = divmod(core, NCORES // B)
        out[b, rc * RCH:(rc + 1) * RCH, :] = res.results[core]["out"]
    return out


# revision 11
# speedup vs baseline: 1.0619x; 1.0619x over previous
"""Trainium2 Bass kernel for ConsistentSelfAttentionTile.

Reference semantics: T=449 overlapping 64-token tiles; each tile attends to
352 KV tokens = 288 sampled (from a 9x replication of the tile) + the tile
itself; outputs overlap-add, then divide by overlap counts.

Algebraic collapse (verified vs the jax reference):
  * rep[:, idx, :] == tile[:, idx % 64, :], so the sampled KV tokens are tile
    rows with integer multiplicities m_t[w] = 1 + #{s : idx[t,s] % 64 == w}.
  * Per-tile Q/K/V are slices of the full-sequence projections, so all
    per-tile 64x64 score blocks are diagonal blocks of one banded 512x512
    score matrix S (band |i-j| <= 63).
  * S itself collapses: S = x G x^T + ones (x) g with G = Wq^T Wk and
    g = bq Wk, both host-precomputed (weight-only folding). Wk/Wq never ship
    to the device and the K projection disappears. bk only shifts rows
    (softmax-invariant): drop.
  * The device computes S TRANSPOSED: ST = x_band (QG)^T with the g bias
    already folded into QG before its transpose, so exp(ST) = E^T directly
    and no E transpose is ever needed (everything downstream consumes E^T).
  * No rowmax subtraction: |S| <= ~50 so e^S spans ~e^{+-50}, comfortably
    inside fp32/bf16 exponent range; every tensor carrying e^S-scaled values
    (E, W, U) is bf16/fp32. A row-constant shift cancels exactly in E/Z, so
    dropping the max is exact.
  * With E = exp(S), Cm[j,t] = m_t[j-t] (banded):
        Z = Cm^T E^T;  W = maskw * (1/Z);  U = Cm W;  out = (E^T o U)^T V
    maskw bakes in the valid-tile mask and the 1/counts overlap division.
  * bv is folded into the output PSUM as a rank-1 ones (x) bv matmul (the
    attention weights sum to 1 after the counts division, so this is exact).
  * Cm^T is transposed on device (PE transpose) instead of shipped.

Sharding: 8 cores = 2 batches x 4 row-chunks of 128 output rows. Each core
computes its 128 rows end-to-end from a 256-column band of the input (no
cross-core communication).

Schedule: 6 dummy matmuls on a zeroed SBUF tile warm the PE clock gate
while the input DMA lands. Input streams over the two HWDGE queues as 2
pieces each, the second piece chained on the first's completion
(concurrent DMAs on one queue interleave across the shared SDMA-engine
pool, so chaining is the only way to give the score-chain bytes priority).
tile_wait_until hints pin the Tile scheduler to the measured arrival times
so it cannot hoist Wv-gated V matmuls ahead of the score chain. The output
is computed and evacuated in column halves so its DMA starts earlier.

"""

import os
import sys

import numpy as np

try:
    import ml_dtypes
except ImportError:
    ml_dtypes = None

for _p in ("/opt/trn_rl_repo",):
    if _p not in sys.path and os.path.isdir(_p):
        sys.path.insert(0, _p)

B, N, C, W = 2, 512, 512, 64
T = N - W + 1          # 449 tiles
RCH = 128              # output rows per core
NCORES = 8
BAND = 256             # per-core j/t band width (columns [r0-64, r0+192))
KC = C // 128          # 4 contraction chunks
JC = BAND // 128       # 2 band chunks

# blob16 layout (2-byte elements per partition), grouped by DMA piece:
#   A1 (sync):   G01 | cm | mw
#   B1 (scalar): xt | G23 | idf | idb   (lands first: QG starts on k2,k3)
#   A2 (sync):   wv01      (chained on A1)
#   B2 (scalar): wv23      (chained on B1)
OFF_G01 = 0                      # [128, 2, 512] fp16  G chunks 0,1
OFF_CM = OFF_G01 + 2 * C         # [128, 2, 256] bf16 (count ints: exact)
OFF_MW = OFF_CM + JC * BAND      # [128, 2, 128] fp16 mask/counts
END_A1 = OFF_MW + JC * RCH
OFF_XT = END_A1                  # [128, 4, 256] fp16  x^T band chunks
OFF_G23 = OFF_XT + KC * BAND     # [128, 2, 512] fp16  G chunks 2,3
OFF_IDF = OFF_G23 + 2 * C        # [128, 128] fp16 identity
OFF_IDB = OFF_IDF + 128          # [128, 128] bf16 identity (bitcast)
END_B1 = OFF_IDB + 128
OFF_WV01 = END_B1                # [128, 2, 512] fp16  Wv^T chunks 0,1
END_A2 = OFF_WV01 + 2 * C
OFF_WV23 = END_A2                # [128, 2, 512] fp16  Wv^T chunks 2,3
F16 = OFF_WV23 + 2 * C

# misc row blob [1, 1152] fp16: g (bq Wk) | bv | ones
MISC_G = 0
MISC_BV = C
MISC_ONES = 2 * C
F_MISC = 2 * C + 128

N_WARM = 9             # dummy matmuls to ungate the PE clock

# Park every live bass semaphore in [232, 256): the walrus end-of-NEFF
# sweep clears [3..255] in five fixed per-engine chunks, and [207..255]
# belongs to Sync -- the engine whose last kernel instruction is the final
# drain. With no exit barrier, the other four engines sweep their (dead)
# chunks concurrently with the output DMA instead of after it.
SEM_BASE = 232


def _patch_sem_range():
    import concourse.env as cenv

    cenv.get_walrus_max_sem_num = lambda: SEM_BASE
    try:
        import concourse.bass as cbass

        cbass.get_walrus_max_sem_num = lambda: SEM_BASE
    except ImportError:
        pass


_CACHE = {}


def _slim_drain_and_barrier(self, tick_clock, wait_clock):
    """Minimal TileContext exit: one Sync drain covering the DMA-queue
    clocks, nothing else. No exit barrier and no explicit sem reset: the
    walrus epilogue sweep (which each engine enters straight after its
    last kernel instruction) zeroes the whole file, and every live sem
    sits in Sync's sweep chunk, which runs after this drain."""
    from concourse.vector_clock import ScopedClock, VectorClock
    from concourse.tile_scheduler import dmasw_start_idx, N_PROCS

    g = tick_clock.global_clock
    dma_clock = VectorClock()
    for idx in range(dmasw_start_idx, N_PROCS):
        t = g.peek_next(idx) - 1
        if t > 0:
            dma_clock.require_at_least(idx, t)
    drain_inst = self.nc.sync.drain()
    wait_clock.add_sem_waits(drain_inst.ins, ScopedClock({None: dma_clock}))
    popped = self.nc._tile_sem_poison_stack.pop()
    assert popped is self._sem_poison


def _build_program():
    _patch_sem_range()
    import concourse.bacc as bacc
    import concourse.mybir as mybir
    import concourse.tile as tile

    fp16 = mybir.dt.float16
    fp32 = mybir.dt.float32
    # Skip Bass's preamble all-engine barrier: all real cross-engine deps
    # here are Tile semaphores, and the one preamble const AP we read (fp32
    # 0.0 for the exp bias) is memset long before the exp fires.
    orig_aeb = bacc.Bacc.all_engine_barrier

    def _noop_aeb(self, *, sem_only=False):
        return None

    bacc.Bacc.all_engine_barrier = _noop_aeb
    try:
        nc = bacc.Bacc("TRN2", target_bir_lowering=False, debug=False)
    finally:
        bacc.Bacc.all_engine_barrier = orig_aeb

    b16_d = nc.declare_dram_parameter("blob16", [128, F16], fp16, isOutput=False)
    misc_d = nc.declare_dram_parameter("misc", [1, F_MISC], fp16, isOutput=False)
    out_d = nc.declare_dram_parameter("out", [RCH, C], fp32, isOutput=True)

    orig_dab = tile.TileContext._drain_and_barrier
    tile.TileContext._drain_and_barrier = _slim_drain_and_barrier
    try:
        _emit_body(nc, tile, mybir, b16_d, misc_d, out_d)
    finally:
        tile.TileContext._drain_and_barrier = orig_dab

    nc.compile()
    return nc


def _emit_body(nc, tile, mybir, b16_d, misc_d, out_d):
    from concourse.tile_rust import add_dep_helper

    fp32 = mybir.dt.float32
    fp16 = mybir.dt.float16
    bf16 = mybir.dt.bfloat16
    COPY = mybir.ActivationFunctionType.Copy

    with tile.TileContext(nc) as tc:

        def at(us):
            """Schedule hint: don't start the instructions in this block
            before `us` microseconds (relative to kernel-body start)."""
            return tc.tile_wait_until(us / 1000.0)

        with (
            tc.tile_pool(name="consts", bufs=1) as consts,
            tc.tile_pool(name="work", bufs=1) as work,
            tc.tile_pool(name="psum", bufs=1, space="PSUM") as psum,
        ):
            b16 = consts.tile([128, F16], fp16)
            misc = consts.tile([1, F_MISC], fp16)
            warm = work.tile([128, 512], bf16)

            # PE warm-up operand: zeros so nothing downstream can see junk.
            nc.gpsimd.memset(warm[:, :], 0)
            nc.gpsimd.dma_start(out=misc[:, :], in_=misc_d[:, :])

            # ---- input DMA: 2 HWDGE queues x 2 pieces, chained in-queue ----
            a1 = nc.sync.dma_start(
                out=b16[:, 0:END_A1], in_=b16_d[:, 0:END_A1])
            b1 = nc.scalar.dma_start(
                out=b16[:, END_A1:END_B1], in_=b16_d[:, END_A1:END_B1])
            a2 = nc.sync.dma_start(
                out=b16[:, END_B1:END_A2], in_=b16_d[:, END_B1:END_A2])
            add_dep_helper(a2.ins, a1.ins, True, "input DMA priority chain")
            b2 = nc.scalar.dma_start(
                out=b16[:, END_A2:F16], in_=b16_d[:, END_A2:F16])
            add_dep_helper(b2.ins, b1.ins, True, "input DMA priority chain")

            # ---- SBUF views ----
            xt_sb = b16[:, OFF_XT:OFF_XT + KC * BAND].rearrange(
                "p (k j) -> p k j", k=KC)
            g01 = b16[:, OFF_G01:OFF_G01 + 2 * C].rearrange(
                "p (k j) -> p k j", k=2)
            g23 = b16[:, OFF_G23:OFF_G23 + 2 * C].rearrange(
                "p (k j) -> p k j", k=2)
            wv01 = b16[:, OFF_WV01:OFF_WV01 + 2 * C].rearrange(
                "p (k j) -> p k j", k=2)
            wv23 = b16[:, OFF_WV23:OFF_WV23 + 2 * C].rearrange(
                "p (k j) -> p k j", k=2)
            g_chunk = [g01[:, 0, :], g01[:, 1, :], g23[:, 0, :], g23[:, 1, :]]
            wv_chunk = [wv01[:, 0, :], wv01[:, 1, :],
                        wv23[:, 0, :], wv23[:, 1, :]]
            idf = b16[:, OFF_IDF:OFF_IDF + 128]
            idb = b16[:, OFF_IDB:OFF_IDB + 128].bitcast(bf16)
            cm_sb = b16[:, OFF_CM:OFF_CM + JC * BAND].bitcast(bf16).rearrange(
                "p (k t) -> p k t", k=JC)
            mw_sb = b16[:, OFF_MW:OFF_MW + JC * RCH].rearrange(
                "p (k r) -> p k r", k=JC)
            g_row = misc[0:1, MISC_G:MISC_G + C]
            bv_row = misc[0:1, MISC_BV:MISC_BV + C]
            ones1 = misc[0:1, MISC_ONES:MISC_ONES + 128]

            # ---- PE clock-gate warm-up (runs while the DMA lands) ----
            ps_aux = psum.tile([128, 512], fp32, tag="ps_aux", bufs=1)
            for _ in range(N_WARM):
                nc.tensor.matmul(
                    ps_aux, lhsT=warm[:, 0:128], rhs=warm[:, :],
                    start=True, stop=True,
                )

            # ---- QG = x G + ones (x) g   [r 128, c2 512] ----
            # accumulation starts on chunks 2,3 (their G rides the
            # earlier-landing B1 piece together with x)
            ps_qg = psum.tile([128, C], fp32, tag="ps_big", bufs=2)
            with at(5.0):
                for i, k in enumerate((2, 3, 0, 1)):
                    nc.tensor.matmul(
                        ps_qg,
                        lhsT=xt_sb[:, k, 64:64 + RCH],
                        rhs=g_chunk[k],
                        start=(i == 0),
                        stop=False,
                    )
                nc.tensor.matmul(
                    ps_qg, lhsT=ones1, rhs=g_row, start=False, stop=True)
            qg_sb = work.tile([128, C], fp16)
            with at(6.3):
                nc.vector.tensor_copy(out=qg_sb, in_=ps_qg)

            # QG^T chunks [c2 128, r 128] via PE transpose (g bias rides
            # along: it was accumulated into QG before the transpose)
            ps_qgt = psum.tile([128, KC, RCH], fp16, tag="ps_t", bufs=1)
            with at(7.0):
                for m in range(KC):
                    nc.tensor.transpose(
                        ps_qgt[:, m, :], qg_sb[:, m * 128:(m + 1) * 128], idf)
            qgt_sb = work.tile([128, KC, RCH], fp16)
            with at(7.7):
                nc.vector.tensor_copy(out=qgt_sb, in_=ps_qgt)

            # ---- transposed scores ST[j 256, r 128] and E^T = exp(ST) ----
            ps_st = psum.tile([128, JC, RCH], fp32, tag="ps_st", bufs=1)
            with at(8.1):
                for jc in range(JC):
                    for k in range(KC):
                        nc.tensor.matmul(
                            ps_st[:, jc, :],
                            lhsT=xt_sb[:, k, jc * 128:(jc + 1) * 128],
                            rhs=qgt_sb[:, k, :],
                            start=(k == 0),
                            stop=(k == KC - 1),
                        )
            et_sb = work.tile([128, JC, RCH], bf16)
            with at(9.3):
                nc.scalar.activation(
                    out=et_sb[:, 0, :], in_=ps_st[:, 0, :],
                    func=mybir.ActivationFunctionType.Exp,
                    bias=0.0, scale=1.0,
                )
                nc.scalar.activation(
                    out=et_sb[:, 1, :], in_=ps_st[:, 1, :],
                    func=mybir.ActivationFunctionType.Exp,
                    bias=0.0, scale=1.0,
                )

            # ---- V[jc][j 128, c 512] = x Wv (bv folded into out) ----
            v_sb = work.tile([128, JC, C], fp16)
            ps_v0 = psum.tile([128, C], fp32, tag="ps_big", bufs=2)
            with at(9.4):
                for k in range(KC):
                    nc.tensor.matmul(
                        ps_v0, lhsT=xt_sb[:, k, 0:128], rhs=wv_chunk[k],
                        start=(k == 0), stop=(k == KC - 1),
                    )

            # Cm^T on device: 4 PE transposes of the cm blocks, slotted
            # into the Tensor gap while Scalar runs the exp
            ps_cmt = psum.tile([128, JC, BAND], bf16, tag="ps_aux", bufs=1)
            with at(9.5):
                for tch in range(JC):
                    for jc in range(JC):
                        nc.tensor.transpose(
                            ps_cmt[:, tch, jc * 128:(jc + 1) * 128],
                            cm_sb[:, jc, tch * 128:(tch + 1) * 128],
                            idb,
                        )
            cmt_sb = work.tile([128, JC, BAND], bf16)
            with at(10.0):
                nc.scalar.activation(out=cmt_sb, in_=ps_cmt, func=COPY)

            # Z[t 128, r 128] per tch = sum_j Cm[j,t] E^T[j,r]
            ps_z = psum.tile([128, JC, RCH], fp32, tag="ps_z", bufs=1)
            with at(9.9):
                for tch in range(JC):
                    for jc in range(JC):
                        nc.tensor.matmul(
                            ps_z[:, tch, :],
                            lhsT=cm_sb[:, jc, tch * 128:(tch + 1) * 128],
                            rhs=et_sb[:, jc, :],
                            start=(jc == 0),
                            stop=(jc == JC - 1),
                        )
            with at(10.8):
                nc.scalar.activation(out=v_sb[:, 0, :], in_=ps_v0, func=COPY)

            # W[t, r] = maskw / Z  (values span e^{+-50}: keep bf16/fp32)
            rz_sb = work.tile([128, JC, RCH], fp32)
            w_sb = work.tile([128, JC, RCH], bf16)
            with at(10.5):
                nc.vector.reciprocal_approx_fast(out=rz_sb, in_=ps_z)
                nc.vector.tensor_mul(w_sb, rz_sb, mw_sb)

            # V jc=1 fills the Tensor gap while Vector runs the W path
            ps_v1 = psum.tile([128, C], fp32, tag="ps_big", bufs=2)
            with at(10.7):
                for k in range(KC):
                    nc.tensor.matmul(
                        ps_v1, lhsT=xt_sb[:, k, 128:256], rhs=wv_chunk[k],
                        start=(k == 0), stop=(k == KC - 1),
                    )

            # U[j 128, r 128] per jc = sum_t Cm^T[t,j] W[t,r];  A = E^T o U
            ps_u = psum.tile([128, JC, RCH], fp32, tag="ps_u", bufs=1)
            with at(11.5):
                for jc in range(JC):
                    for tch in range(JC):
                        nc.tensor.matmul(
                            ps_u[:, jc, :],
                            lhsT=cmt_sb[:, tch, jc * 128:(jc + 1) * 128],
                            rhs=w_sb[:, tch, :],
                            start=(tch == 0),
                            stop=(tch == JC - 1),
                        )
            with at(12.1):
                nc.scalar.activation(out=v_sb[:, 1, :], in_=ps_v1, func=COPY)
            a_sb = work.tile([128, JC, RCH], fp16)
            with at(12.1):
                nc.vector.tensor_mul(a_sb, ps_u, et_sb)

            # out[r 128, c 512] = sum_j A[j,r] V[j,c] + ones (x) bv,
            # computed and evacuated in column halves so the output DMA
            # starts while the second half is still in the PE
            ps_o = psum.tile([128, C], fp32, tag="ps_big", bufs=2)
            o_sb = work.tile([128, C], fp32)
            with at(12.6):
                for jc in range(JC):
                    nc.tensor.matmul(
                        ps_o,
                        lhsT=a_sb[:, jc, :],
                        rhs=v_sb[:, jc, :],
                        start=(jc == 0),
                        stop=False,
                    )
                nc.tensor.matmul(
                    ps_o, lhsT=ones1, rhs=bv_row, start=False, stop=True)
            with at(13.4):
                nc.vector.tensor_copy(out=o_sb[:, 0:256], in_=ps_o[:, 0:256])
                nc.sync.dma_start(out=out_d[:, 0:256], in_=o_sb[:, 0:256])
            with at(14.0):
                nc.scalar.activation(
                    out=o_sb[:, 256:512], in_=ps_o[:, 256:512], func=COPY)
                nc.scalar.dma_start(
                    out=out_d[:, 256:512], in_=o_sb[:, 256:512])


def _pack128(arr):
    """[n*128, f] row-chunked -> [128, n*f] (chunk-major along free axis)."""
    n = arr.shape[0] // 128
    return np.ascontiguousarray(
        arr.reshape(n, 128, -1).transpose(1, 0, 2).reshape(128, -1)
    )


def _host_prep(image_features, Wq, bq, Wk, bk, Wv, bv, sample_idx):
    """Build the 8 per-core input blobs (pure index/layout work plus
    weight-only constant folding)."""
    x = np.asarray(image_features, np.float32)
    sample_idx = np.asarray(sample_idx)
    Wq = np.asarray(Wq, np.float32)
    Wk = np.asarray(Wk, np.float32)
    Wv = np.asarray(Wv, np.float32)
    bq = np.asarray(bq, np.float32)
    bv = np.asarray(bv, np.float32)

    # score-collapse: S = x G x^T + ones (x) g   (bk drops: row shift)
    G = (Wq.T @ Wk).astype(np.float16)
    g = (bq @ Wk).astype(np.float16)

    # per-tile multiplicities -> banded count matrix Cm[j, t] = m_t[j - t]
    mod = (sample_idx % W).astype(np.int64)                  # [T, S]
    m = np.zeros((T, W), np.float32)
    np.add.at(m, (np.arange(T)[:, None], mod), 1.0)
    m += 1.0
    Cm = np.zeros((N, N), np.float32)
    rows = np.arange(T)
    for w in range(W):
        Cm[rows + w, rows] = m[:, w]

    pos = np.arange(N)
    counts = (np.minimum(pos, N - W) - np.maximum(pos - W + 1, 0) + 1)

    # padded versions for uniform band slicing
    XTp = np.zeros((B, C, N + 2 * 64), np.float16)
    for b in range(B):
        XTp[b, :, 64:64 + N] = x[b].T.astype(np.float16)
    Cmp = np.zeros((N + 2 * 64, N + 2 * 64), np.float32)
    Cmp[64:64 + N, 64:64 + N] = Cm

    g_p = _pack128(G.astype(np.float16))                     # [128, 4*512]
    wvt_p = _pack128(Wv.T.astype(np.float16))                # [128, 4*512]

    misc = np.zeros((1, F_MISC), np.float16)
    misc[0, MISC_G:MISC_G + C] = g
    misc[0, MISC_BV:MISC_BV + C] = bv
    misc[0, MISC_ONES:MISC_ONES + 128] = 1.0

    in_maps = []
    for core in range(NCORES):
        b, rc = divmod(core, NCORES // B)
        r0 = rc * RCH
        xt = XTp[b, :, r0:r0 + BAND]
        cm = np.ascontiguousarray(Cmp[r0:r0 + BAND, r0:r0 + BAND])
        # all-zero columns (padded t) would give Z=0 -> inf*0 = NaN on
        # device; a diagonal 1 keeps Z finite there and is masked out of W
        zero_cols = ~cm.any(axis=0)
        cm[zero_cols, zero_cols] = 1.0
        tl = np.arange(BAND)
        rl = np.arange(RCH)
        tg = r0 - 64 + tl
        rg = r0 + rl
        d = rg[None, :] - tg[:, None]
        valid = (d >= 0) & (d <= W - 1) & (tg[:, None] >= 0) & (tg[:, None] <= T - 1)
        maskw = np.where(
            valid, 1.0 / counts[rg][None, :], 0.0
        ).astype(np.float16)

        b16 = np.zeros((128, F16), np.float16)
        b16[:, OFF_XT:OFF_XT + KC * BAND] = _pack128(xt)
        b16[:, OFF_G01:OFF_G01 + 2 * C] = g_p[:, 0:2 * C]
        b16[:, OFF_G23:OFF_G23 + 2 * C] = g_p[:, 2 * C:4 * C]
        b16[:, OFF_WV01:OFF_WV01 + 2 * C] = wvt_p[:, 0:2 * C]
        b16[:, OFF_WV23:OFF_WV23 + 2 * C] = wvt_p[:, 2 * C:4 * C]
        b16[:, OFF_MW:OFF_MW + JC * RCH] = _pack128(maskw)
        b16[:, OFF_IDF:OFF_IDF + 128] = np.eye(128, dtype=np.float16)
        # bf16-bit segments written through a uint16 view of the fp16 buffer
        b16v = b16.view(np.uint16)
        b16v[:, OFF_CM:OFF_CM + JC * BAND] = _pack128(
            cm.astype(ml_dtypes.bfloat16)).view(np.uint16)
        b16v[:, OFF_IDB:OFF_IDB + 128] = np.eye(
            128, dtype=ml_dtypes.bfloat16).view(np.uint16)

        in_maps.append({"blob16": b16, "misc": misc})
    return in_maps


def run_on_cores(in_maps, trace=False, trace_cores=None):
    from concourse.bass_utils import run_bass_kernel_spmd

    if "nc" not in _CACHE:
        _CACHE["nc"] = _build_program()
    nc = _CACHE["nc"]
    return run_bass_kernel_spmd(
        nc, in_maps, list(range(NCORES)), trace=trace,
        trace_cores=(trace_cores or [0]) if trace else None,
    )


def kernel(image_features, Wq, bq, Wk, bk, Wv, bv, sample_idx):
    in_maps = _host_prep(image_features, Wq, bq, Wk, bk, Wv, bv, sample_idx)
    res = run_on_cores(in_maps, trace=False)
    out = np.empty((B, N, C), np.float32)
    for core in range(NCORES):
        b, rc = divmod(core, NCORES // B)
        out[b, rc * RCH:(rc + 1) * RCH, :] = res.results[core]["out"]
    return out
